# revision 4
# baseline (speedup 1.0000x reference)
"""Trainium2 Bass kernel for nn_Attention_Rel_Scl (B=8,S=1024,E=1024,H=16).

Data-parallel over batch: one batch element per NeuronCore (8 cores).

Key ideas:
  - All matmuls fp16 (1 cyc/row on the PE; ~4e-4 end-to-end rel err).
  - Attention computed transposed (attnT[j, i]) so the softmax denominator
    is a PE column-sum (matmul with a ones stationary) and P@V needs no
    transposes of the 16M-element attention tensor.
  - No max-subtraction in softmax (logits ~ N(0, 0.1) => exp safe);
    mathematically identical to jax.nn.softmax.
  - The (h,S,S) relative bias is never materialized. Verified identity:
        bias[h,i,j] = flat[(15360 - 1024*h) + 1024*(i%16) - 16*(i//16) + j]
    with flat = rel_table.reshape(-1) (clip in the reference never fires).
    Post-softmax bias distributes: out = (P@v)/s + bias@v, and bias@v is a
    plain PE matmul whose moving operand is a strided view into a diagonal
    SBUF buffer T2[p, w] = flat[p + w] (no bias DMA traffic at all).
  - Query rows are processed in permuted order sigma(f) = 16*(63 - f%64) +
    f//64, which makes the T2 view affine; the host un-permutes output rows.
  - LayerNorm fused at the end: PE transposes -> bn_stats/bn_aggr -> apply.
"""

import sys

if "/opt/trn_rl_repo" not in sys.path:
    sys.path.insert(0, "/opt/trn_rl_repo")

import numpy as np

B, S, E, H = 8, 1024, 1024, 16
D = E // H          # 64 head dim
P = 128             # partitions
G = H // 2          # 8 head pairs
NBLK = S // P       # 8 key blocks
KBLK = E // P       # 8 contraction blocks
EPS = 1e-3
SCALE = float(E) ** -0.5
FLAT = (2 * S - 1) * H   # 32752
T2W = 32625              # max free offset 32624 (+p<=127 -> 32751 = FLAT-1)

# processing position f -> true query row (within each batch row space)
_f = np.arange(S)
SIGMA = 16 * (63 - _f % 64) + _f // 64
# stored partition p -> row offset within its 128-row output block
_p = np.arange(P)
ROWMAP = 16 * (7 - _p % 8) + _p // 8

_BUILT = {}


def _build(trivial_ln: bool):
    import concourse.bass as bass
    import concourse.tile as tile
    from concourse import bacc, mybir
    from concourse.masks import make_identity
    from contextlib import ExitStack

    fp16 = mybir.dt.float16
    fp32 = mybir.dt.float32
    Exp = mybir.ActivationFunctionType.Exp
    Sqrt = mybir.ActivationFunctionType.Sqrt
    mult = mybir.AluOpType.mult
    add = mybir.AluOpType.add
    sub = mybir.AluOpType.subtract

    nc = bacc.Bacc("TRN2", target_bir_lowering=False, debug=False,
                   num_devices=8)

    x16 = nc.dram_tensor("x16", [S, E], fp16, kind="ExternalInput").ap()
    xp16 = nc.dram_tensor("xp16", [S, E], fp16, kind="ExternalInput").ap()
    wq16 = nc.dram_tensor("wq16", [E, E], fp16, kind="ExternalInput").ap()
    wk16 = nc.dram_tensor("wk16", [E, E], fp16, kind="ExternalInput").ap()
    wv16 = nc.dram_tensor("wv16", [E, E], fp16, kind="ExternalInput").ap()
    flat16 = nc.dram_tensor("flat16", [FLAT], fp16, kind="ExternalInput").ap()
    if not trivial_ln:
        gam = nc.dram_tensor("gamma", [1, E], fp32, kind="ExternalInput").ap()
        bet = nc.dram_tensor("beta", [1, E], fp32, kind="ExternalInput").ap()
    out = nc.dram_tensor("out", [S, E], fp32, kind="ExternalOutput").ap()

    with tile.TileContext(nc) as tc, ExitStack() as ctx:
        persist = ctx.enter_context(tc.tile_pool(name="persist", bufs=1))
        T2 = persist.tile([P, T2W], fp16, name="T2")        # 63.7 KB/p
        QT = persist.tile([P, G, S], fp16, name="QT")       # 16 KB/p
        KT = persist.tile([P, G, S], fp16, name="KT")       # 16 KB/p
        V = persist.tile([P, NBLK, E], fp16, name="V")      # 16 KB/p
        ones16 = persist.tile([P, D], fp16, name="ones16")
        ident = persist.tile([P, P], fp32, name="ident")
        epsT = persist.tile([P, 1], fp32, name="epsT")

        nc.vector.memset(ones16, 1.0)
        nc.vector.memset(epsT, EPS)
        make_identity(nc, ident)

        # T2[p, w] = flat[p + w]  (one overlapping-read DMA, ~8.3 MB)
        nc.sync.dma_start(
            out=T2,
            in_=bass.AP(tensor=flat16.tensor, offset=0,
                        ap=[[1, P], [1, T2W]]),
        )

        if not trivial_ln:
            gamT = persist.tile([P, E], fp32, name="gamT")
            betT = persist.tile([P, E], fp32, name="betT")
            nc.sync.dma_start(
                out=gamT,
                in_=bass.AP(tensor=gam.tensor, offset=0, ap=[[0, P], [1, E]]),
            )
            nc.sync.dma_start(
                out=betT,
                in_=bass.AP(tensor=bet.tensor, offset=0, ap=[[0, P], [1, E]]),
            )

        # ============ Stage 1: xT / xTp, Q^T, K^T, V projections ==========
        with tc.tile_pool(name="s1fix", bufs=1) as s1fix, \
             tc.tile_pool(name="wpool", bufs=3) as wpool, \
             tc.tile_pool(name="ps1", bufs=4, space="PSUM") as ps1:
            xT = s1fix.tile([P, KBLK, S], fp16, name="xT")
            xTp = s1fix.tile([P, KBLK, S], fp16, name="xTp")
            wv_sb = s1fix.tile([P, KBLK, E], fp16, name="wv_sb")
            for blk in range(KBLK):
                nc.sync.dma_start_transpose(
                    xT[:, blk, :], x16[:, blk * P:(blk + 1) * P])
                nc.sync.dma_start_transpose(
                    xTp[:, blk, :], xp16[:, blk * P:(blk + 1) * P])
            nc.sync.dma_start(
                out=wv_sb,
                in_=wv16.rearrange("(kb kp) e -> kp kb e", kp=P),
            )

            # QT[d2, g, f]: lhsT = Wq[:, pair-cols] (streamed), rhs = xTp
            # KT[d2, g, j]: same with Wk / xT
            for wdram, dst, rhs_src, eng in (
                (wq16, QT, xTp, nc.scalar), (wk16, KT, xT, nc.scalar),
            ):
                for g in range(G):
                    wt = wpool.tile([P, KBLK, P], fp16, tag="wtile")
                    nc.sync.dma_start(
                        out=wt,
                        in_=wdram.rearrange("(kb kp) e -> kp kb e", kp=P)[
                            :, :, g * P:(g + 1) * P],
                    )
                    for ic in range(2):
                        pt = ps1.tile([P, 512], fp32, tag="ps1t")
                        for kb in range(KBLK):
                            nc.tensor.matmul(
                                pt, wt[:, kb, :],
                                rhs_src[:, kb, ic * 512:(ic + 1) * 512],
                                start=(kb == 0), stop=(kb == KBLK - 1),
                            )
                        eng.copy(dst[:, g, ic * 512:(ic + 1) * 512], pt)

            # V[j, jb, e']: lhsT = xT[:, kb, j-block], rhs = Wv rows
            for jb in range(NBLK):
                for ic in range(2):
                    pt = ps1.tile([P, 512], fp32, tag="ps1t")
                    for kb in range(KBLK):
                        nc.tensor.matmul(
                            pt, xT[:, kb, jb * P:(jb + 1) * P],
                            wv_sb[:, kb, ic * 512:(ic + 1) * 512],
                            start=(kb == 0), stop=(kb == KBLK - 1),
                        )
                    nc.vector.tensor_copy(
                        V[:, jb, ic * 512:(ic + 1) * 512], pt)

        # ============ Stage 2 + 3 scope =================================
        with tc.tile_pool(name="s23", bufs=1) as s23:
            outT = s23.tile([P, G, S], fp32, name="outT")   # 32 KB/p

            # ---- Stage 2: attention per head pair ----
            with tc.tile_pool(name="expp", bufs=2) as expp, \
                 tc.tile_pool(name="sr", bufs=2) as srpool, \
                 tc.tile_pool(name="psA", bufs=3, space="PSUM") as psA, \
                 tc.tile_pool(name="psS", bufs=2, space="PSUM") as psS, \
                 tc.tile_pool(name="psP", bufs=2, space="PSUM") as psP, \
                 tc.tile_pool(name="psB", bufs=1, space="PSUM") as psB:
                for g in range(G):
                    eP = [expp.tile([P, NBLK, S], fp16, tag="ept",
                                    name=f"eP{g}_{h_}")
                          for h_ in range(2)]
                    for half in range(2):
                        lo = D * half
                        for J in range(NBLK):
                            for ic in range(2):
                                pa = psA.tile([P, 512], fp32, tag="pat")
                                nc.tensor.matmul(
                                    pa,
                                    KT[lo:lo + D, g, J * P:(J + 1) * P],
                                    QT[lo:lo + D, g, ic * 512:(ic + 1) * 512],
                                    start=True, stop=True,
                                )
                                nc.scalar.activation(
                                    out=eP[half][:, J, ic * 512:(ic + 1) * 512],
                                    in_=pa, func=Exp, scale=SCALE,
                                )
                    for ic in range(2):
                        ps = psS.tile([P, 512], fp32, tag="pst")
                        pp = psP.tile([P, 512], fp32, tag="ppt")
                        pb = psB.tile([P, 512], fp32, tag="pbt")
                        for half in range(2):
                            hh = 2 * g + half
                            lo = D * half
                            c_h = 15360 - 1024 * hh
                            for J in range(NBLK):
                                rhs = eP[half][:, J, ic * 512:(ic + 1) * 512]
                                nc.tensor.matmul(
                                    ps[lo:lo + D, :], ones16, rhs,
                                    start=(J == 0), stop=(J == NBLK - 1),
                                    skip_group_check=True,
                                )
                                nc.tensor.matmul(
                                    pp[lo:lo + D, :],
                                    V[:, J, hh * D:(hh + 1) * D], rhs,
                                    start=(J == 0), stop=(J == NBLK - 1),
                                    skip_group_check=True,
                                )
                                t2v = bass.AP(
                                    tensor=T2.tensor,
                                    offset=T2.offset + c_h + 8192 * ic + P * J,
                                    ap=[T2.ap[0], [1024, 8], [16, 64]],
                                )
                                nc.tensor.matmul(
                                    pb[lo:lo + D, :],
                                    V[:, J, hh * D:(hh + 1) * D], t2v,
                                    start=(J == 0), stop=(J == NBLK - 1),
                                    skip_group_check=True,
                                )
                        srec = srpool.tile([P, 512], fp32, tag="srt")
                        nc.vector.reciprocal(srec, ps)
                        dstc = outT[:, g, ic * 512:(ic + 1) * 512]
                        nc.vector.tensor_tensor(dstc, pp, srec, mult)
                        nc.vector.tensor_tensor(dstc, dstc, pb, add)

            # ---- Stage 3: transpose + LayerNorm + store ----
            with tc.tile_pool(name="ln", bufs=3) as ln, \
                 tc.tile_pool(name="psL", bufs=2, space="PSUM") as psL:
                for T in range(NBLK):
                    pl = psL.tile([P, E], fp32, tag="plt")
                    tmp = ln.tile([P, E], fp32, tag="tmpT")
                    for g in range(G):
                        # gather block-T columns contiguously (stationary
                        # matmul operands must have a single free dim)
                        src = bass.AP(
                            tensor=outT.tensor,
                            offset=outT.offset + g * S + (56 - 8 * T),
                            ap=[outT.ap[0], [64, 16], [1, 8]],
                        )
                        nc.vector.tensor_copy(tmp[:, g * P:(g + 1) * P], src)
                        nc.tensor.matmul(
                            pl[:, g * P:(g + 1) * P],
                            tmp[:, g * P:(g + 1) * P], ident,
                            is_transpose=True, skip_group_check=True,
                        )
                    stats = ln.tile([P, 2, 6], fp32, tag="stats")
                    mv = ln.tile([P, 2], fp32, tag="mv")
                    for c in range(2):
                        nc.vector.bn_stats(
                            stats[:, c, :], pl[:, c * 512:(c + 1) * 512])
                    nc.vector.bn_aggr(mv, stats)
                    rstd = ln.tile([P, 1], fp32, tag="rstd")
                    murs = ln.tile([P, 1], fp32, tag="murs")
                    nc.scalar.activation(out=rstd, in_=mv[:, 1:2],
                                         func=Sqrt, bias=epsT, scale=1.0)
                    nc.vector.reciprocal(rstd, rstd)
                    nc.vector.tensor_tensor(murs, mv[:, 0:1], rstd, mult)
                    of = ln.tile([P, E], fp32, tag="of")
                    nc.vector.tensor_scalar(of, pl, rstd, murs, op0=mult,
                                            op1=sub)
                    if not trivial_ln:
                        nc.vector.tensor_tensor(of, of, gamT, mult)
                        nc.vector.tensor_tensor(of, of, betT, add)
                    nc.sync.dma_start(out[T * P:(T + 1) * P, :], of)

    nc.compile()
    return nc


def get_nc(trivial_ln: bool = True):
    if trivial_ln not in _BUILT:
        _BUILT[trivial_ln] = _build(trivial_ln)
    return _BUILT[trivial_ln]


def make_in_maps(inputs):
    x = np.asarray(inputs["x"])
    rel = np.asarray(inputs["rel_table"])
    gamma = np.asarray(inputs["gamma"])
    beta = np.asarray(inputs["beta"])
    trivial_ln = bool(np.all(gamma == 1.0) and np.all(beta == 0.0))

    x16 = x.astype(np.float16)
    xp16 = np.ascontiguousarray(x16[:, SIGMA, :])
    wq16 = np.asarray(inputs["Wq"]).astype(np.float16)
    wk16 = np.asarray(inputs["Wk"]).astype(np.float16)
    wv16 = np.asarray(inputs["Wv"]).astype(np.float16)
    flat16 = np.ascontiguousarray(rel.reshape(-1).astype(np.float16))

    in_maps = []
    for b in range(x.shape[0]):
        m = {"x16": np.ascontiguousarray(x16[b]), "xp16": xp16[b],
             "wq16": wq16, "wk16": wk16, "wv16": wv16, "flat16": flat16}
        if not trivial_ln:
            m["gamma"] = gamma.reshape(1, E).astype(np.float32)
            m["beta"] = beta.reshape(1, E).astype(np.float32)
        in_maps.append(m)
    return in_maps, trivial_ln


def unpermute(raw):
    """raw: (..., S, E) with permuted rows -> natural row order."""
    unperm = (np.arange(0, S, P)[:, None] + ROWMAP[None, :]).reshape(-1)
    fixed = np.empty_like(raw)
    fixed[..., unperm, :] = raw
    return fixed


def kernel(**inputs) -> np.ndarray:
    from concourse import bass_utils

    in_maps, trivial_ln = make_in_maps(inputs)
    nc = get_nc(trivial_ln)
    res = bass_utils.run_bass_kernel_spmd(nc, in_maps,
                                          core_ids=list(range(len(in_maps))))
    outs = np.stack([r["out"] for r in res.results])
    return unpermute(outs).astype(np.float32)
